# revision 40
# baseline (speedup 1.0000x reference)
"""Multi-head attention kernel for Trainium2, distributed over 8 NeuronCores.

Problem: x[8,1,2048,384] @ W_qkv[384,1152] -> 8-head attention (dk=48,
softmax scale 1/sqrt(2048)) -> @ W_o[384,384] + b_o.

Sharding: batch (b=8) data-parallel, one batch element per core. No
collectives.

v3 design (ACT exp is the bottleneck engine; PE attention cost collapsed
via fp8 DoubleRow PV):
  - QK and the exp stream are unchanged from v2: per (pair, c5, t) one
    sAB [128,1024] PSUM tile (head A cols 0:512, head B 512:1024), one
    [128,1024] ACT Exp -> bf16 ptb. 256 ACT ops total.
  - exp(s) quantizes poorly to fp8 (values cluster at 1 where e4m3 step is
    1/8), so PV uses the expm1 decomposition: O = sum_m (w_m - 1) v_m +
    sum_m v_m.  Pool (otherwise idle) computes pt8 = ptb - 1 -> fp8 with a
    strided output AP that lands key-pair-interleaved: pt8[b] is a
    [128,2048] fp8 tile, word head*1024 + 2q + (t%2) for block b = keys of
    nt 2b,2b+1.  PV is then ONE fp8 DoubleRow matmul per (block, head):
    lhsT = v8[b][head] [128 pairs, 2, 64], rhs = pt8 pairs [128, 2, 512],
    256-key contraction at 0.5 cyc/row: 262k -> 66k PE cycles.
  - v8 = v in fp8 with the same ones-column trick as v2 (even head: data
    cols 0:48, ones 48:64 -> Z on o row 63; odd: ones 0:16, data 16:64 ->
    Z on row 64), slot = nt parity, so the stage-A copies stay
    partition-aligned.
  - Corrections (exact, cancel both the -1 shift and v's fp8 noise):
    cs[c] = sum_n v[n,c] computed as (sum_n x) @ W_v: DVE row-reduces xT,
    3 tiny matmuls vs W_v, then per-pair [128,1] fp32 csp tiles via DMA
    partition-scatter.  The Z rows need +2048 (ones-column colsum): a DVE
    tensor_scalar_add into ztmp before the reciprocals.  The data rows get
    cs added inside the existing normalization multiply, upgraded to
    scalar_tensor_tensor: attn = (o + csp) * (1/Z) -- same DVE cost as v2.
  - Everything else (projection windows interleaved into the attention
    stream, fc_o per q-range, per-pair shuffled W_o, 1/Z DRAM-bounce
    broadcast, pools shared across reps) is unchanged from v2.

Measured: HW (8-core SPMD, rep-differenced) v2 baseline 366us; sim (CoreSim
cost model) v2 327us with PE 282us / ACT 267us busy -> v3 targets the ACT
floor (256 * ~1.04us) with PE ~205us.  Max rel err vs fp32 reference
5.5e-3 in numpy modeling (gate 2e-2).
"""

import numpy as np

import concourse.bass as bass
import concourse.mybir as mybir
import concourse.tile as tile
from concourse import bacc
from concourse.bass_utils import run_bass_kernel_spmd
from concourse.masks import make_identity

F32 = mybir.dt.float32
BF16 = mybir.dt.bfloat16
FP8 = mybir.dt.float8e4
AF = mybir.ActivationFunctionType
OP = mybir.AluOpType
DR = mybir.MatmulPerfMode.DoubleRow

N = 2048          # sequence length per core
D = 384           # d_model
H = 8             # heads
DK = 48           # head dim
NCORES = 8
SCALE = 1.0 / float(np.sqrt(N))  # reference scales by sqrt(seq), not sqrt(dk)

NT = N // 128     # 16 n-tiles of 128
NB = NT // 2      # 8 blocks of 256 keys
DT3 = D // 128    # 3 d-model chunks
VW = 64           # v8 columns per head (48 data + ones/zeros filler)


def build_nc(reps=1, stages="absepnf"):
    nc = bacc.Bacc(debug=False)
    x = nc.declare_dram_parameter("x", [N, D], F32, isOutput=False).ap()
    w_qkv = nc.declare_dram_parameter("W_qkv", [D, 3 * D], F32, isOutput=False).ap()
    w_o = nc.declare_dram_parameter("W_o", [D, D], F32, isOutput=False).ap()
    b_o = nc.declare_dram_parameter("b_o", [D], F32, isOutput=False).ap()
    out = nc.declare_dram_parameter("out", [N, D], F32, isOutput=True).ap()

    with tile.TileContext(nc) as tc:
        _emit(nc, tc, x, w_qkv, w_o, b_o, out, reps, stages)
    nc.compile()
    return nc


def _emit(nc, tc, x, w_qkv, w_o, b_o, out, reps=1, stages="absepnf"):
    from contextlib import ExitStack

    ctx = ExitStack()
    with ctx:
        persist = ctx.enter_context(tc.tile_pool(name="persist", bufs=1))

        # --- constants -----------------------------------------------------
        ident = persist.tile([128, 128], BF16)
        make_identity(nc, ident)
        onesb = persist.tile([128, 1], BF16)
        nc.gpsimd.memset(onesb, 1.0)

        # W_qkv as 3 d-chunk tiles [128, 1152] bf16 (needed by the first
        # v-projection, so loaded up front)
        wqkv_sb = []
        wstage = ctx.enter_context(tc.tile_pool(name="wstage", bufs=2))
        for dc in range(DT3):
            w_stage = wstage.tile([128, 3 * D], F32)
            nc.sync.dma_start(out=w_stage, in_=w_qkv[dc * 128 : (dc + 1) * 128, :])
            w_t = persist.tile([128, 3 * D], BF16, tag=f"wqkv{dc}", name=f"wqkv{dc}")
            nc.gpsimd.tensor_copy(w_t, w_stage)
            wqkv_sb.append(w_t)

        # Per-pair W_o tiles (rows 0:48 = head 2p, 80:128 = head 2p+1, rows
        # 48:80 zero) and the b_o broadcast are not needed until fc_o --
        # allocate handles now, DMAs deferred until after stage A.
        wo_sb = [
            persist.tile([128, D], BF16, tag=f"wo{p}", name=f"wo{p}")
            for p in range(H // 2)
        ]
        b_bcast = persist.tile([128, D], F32)

        def emit_weight_tail():
            wo_bf = []
            for dc in range(DT3):
                w_stage = wstage.tile([128, D], F32, tag="wostage")
                nc.sync.dma_start(out=w_stage, in_=w_o[dc * 128 : (dc + 1) * 128, :])
                w_t = persist.tile([128, D], BF16, tag=f"wob{dc}", name=f"wob{dc}")
                nc.vector.tensor_copy(w_t, w_stage)
                wo_bf.append(w_t)
            for p in range(H // 2):
                wt = wo_sb[p]
                nc.gpsimd.memset(wt, 0.0)
                for dst0, src0 in ((0, 96 * p), (80, 96 * p + 48)):
                    done = 0
                    while done < 48:
                        srow = src0 + done
                        t_i, t_r = srow // 128, srow % 128
                        n_r = min(48 - done, 128 - t_r)
                        nc.sync.dma_start(
                            out=wt[dst0 + done : dst0 + done + n_r, :],
                            in_=wo_bf[t_i][t_r : t_r + n_r, :],
                        )
                        done += n_r
            b_src = bass.AP(tensor=b_o.tensor, offset=0, ap=[[0, 128], [1, D]])
            nc.sync.dma_start(out=b_bcast, in_=b_src)

        # --- persistent arrays ---------------------------------------------
        xT = [
            persist.tile([128, N], BF16, tag=f"xT{dc}", name=f"xT{dc}")
            for dc in range(DT3)
        ]
        # q/k in fp8 DoubleRow pair layout: [128, 2*N] fp8, partition rows
        # 0:24 = head A (dk dim d at (p, i) = (d//2, d%2)), rows 64:88 =
        # head B; free byte = i*N + n.
        q_pack = [
            persist.tile([128, 2 * N], FP8, tag=f"qp{p}", name=f"qp{p}")
            for p in range(H // 2)
        ]
        k_pack = [
            persist.tile([128, 2 * N], FP8, tag=f"kp{p}", name=f"kp{p}")
            for p in range(H // 2)
        ]
        # v8[parity][b]: [128, H/2, 2, VW] fp8 for the EVEN heads (DR PV);
        # partition = key pair index p of 256-key block b, slot = nt parity
        # (key = 256b + 128*slot + p); data cols 0:48, ones 48:64 (Z -> o
        # row 63).  vb16[parity][nt]: [128, H/2, VW] bf16 for the ODD heads
        # (plain bf16 PV straight from ptb; DR can't write dst partition 64);
        # ones 0:16, data 16:64 (Z -> o row 64).
        v8 = [
            [
                persist.tile([128, (H // 2) * 2 * VW], FP8, tag=f"v8_{par}_{b}",
                             name=f"v8_{par}_{b}")
                for b in range(NB)
            ]
            for par in range(2)
        ]
        vb16 = [
            [
                persist.tile([128, (H // 2) * VW], BF16, tag=f"vb_{par}_{t}",
                             name=f"vb_{par}_{t}")
                for t in range(NT)
            ]
            for par in range(2)
        ]
        # per-pair attn tiles [128, N] bf16; rows 0:48 head 2p (normalized),
        # rows 80:128 head 2p+1, rows 48:80 garbage (W_o rows zero there).
        attn_p = [
            persist.tile([128, N], BF16, tag=f"at{p}", name=f"at{p}")
            for p in range(H // 2)
        ]

        for par in range(min(2, reps)):
            for b in range(NB):
                vph = v8[par][b].rearrange("p (hp i c) -> p hp i c",
                                           i=2, c=VW)
                nc.gpsimd.memset(vph[:, :, :, 48:64], 1.0)
            for t in range(NT):
                vbh = vb16[par][t].rearrange("p (hp c) -> p hp c", c=VW)
                nc.gpsimd.memset(vbh[:, :, 0:16], 1.0)

        pools = {
            "scratch": ctx.enter_context(
                tc.tile_pool(name="scratch", bufs=2, space="PSUM")
            ),
            "spsum": ctx.enter_context(tc.tile_pool(name="spsum", bufs=2, space="PSUM")),
            "opsum": ctx.enter_context(tc.tile_pool(name="opsum", bufs=2, space="PSUM")),
            "ptpool": ctx.enter_context(tc.tile_pool(name="ptpool", bufs=6)),
            "pt8pool": ctx.enter_context(tc.tile_pool(name="pt8pool", bufs=4)),
            "zpool": ctx.enter_context(tc.tile_pool(name="zpool", bufs=2)),
            "ztpool": ctx.enter_context(tc.tile_pool(name="ztpool", bufs=2)),
            "zbpool": ctx.enter_context(tc.tile_pool(name="zbpool", bufs=2)),
            "zdpool": ctx.enter_context(
                tc.tile_pool(name="zdpool", bufs=2, space="DRAM")
            ),
            "cspool": ctx.enter_context(tc.tile_pool(name="cspool", bufs=2)),
            "t8p": ctx.enter_context(tc.tile_pool(name="t8p", bufs=4)),
            "csx": ctx.enter_context(tc.tile_pool(name="csx", bufs=2)),
            "xload": ctx.enter_context(tc.tile_pool(name="xload", bufs=6)),
            "xbp": ctx.enter_context(tc.tile_pool(name="xbp", bufs=4)),
            "fout": ctx.enter_context(tc.tile_pool(name="fout", bufs=3)),
        }

        carry = None
        for rep in range(reps):
            carry = _emit_pipeline(
                nc, tc, x, out, ident, onesb, wqkv_sb, wo_sb, b_bcast,
                xT, q_pack, k_pack, v8[rep % 2], vb16[rep % 2], attn_p, pools,
                rep, stages,
                deferred=emit_weight_tail if rep == 0 else None,
                carry_fc=carry,
                preload_next=(rep + 1 < reps and "a" in stages
                              and "s" in stages),
                stageA_preloaded=(rep > 0 and "s" in stages),
            )


def _emit_pipeline(
    nc, tc, x, out, ident, onesb, wqkv_sb, wo_sb, b_bcast,
    xT, q_pack, k_pack, v8, vb16, attn_p, pools, rep, stages="absepnf",
    deferred=None, carry_fc=None, preload_next=False, stageA_preloaded=False,
):
    scratch = pools["scratch"]
    spsum = pools["spsum"]
    opsum = pools["opsum"]
    ptpool = pools["ptpool"]
    pt8pool = pools["pt8pool"]
    zpool = pools["zpool"]
    ztpool = pools["ztpool"]
    zbpool = pools["zbpool"]
    zdpool = pools["zdpool"]
    cspool = pools["cspool"]
    t8pool = pools["t8p"]
    csxpool = pools["csx"]
    xload = pools["xload"]
    xbp = pools["xbp"]
    fout = pools["fout"]

    # --- q/k projection windows ---------------------------------------
    proj_state = {}

    def emit_proj_step(pair, qk, c4, dc, ns=0):
        # one dc-step (a single 96-row strip matmul: heads 2p/2p+1 have
        # contiguous W_qkv columns) of a projection window
        dest = q_pack[pair] if qk == 0 else k_pack[pair]
        base = qk + 96 * pair
        cs = slice(c4 * 512, (c4 + 1) * 512)
        if dc == 0:
            proj_state[(ns, pair, qk, c4)] = scratch.tile(
                [128, 512], F32, tag="scr",
                name=f"pp{rep}_{ns}_{pair}_{qk}_{c4}"
            )
        pp = proj_state[(ns, pair, qk, c4)]
        nc.tensor.matmul(
            pp[0:96, :],
            wqkv_sb[dc][:, base : base + 96],
            xT[dc][:, cs],
            start=(dc == 0), stop=(dc == DT3 - 1),
            skip_group_check=True,
        )
        if dc == DT3 - 1:
            # fp8-convert, then DMA partition-remap d -> (d//2, d%2) into
            # the DoubleRow pair layout (head A rows 0:48 -> partitions
            # 0:24, head B rows 48:96 -> partitions 64:88).
            t8 = t8pool.tile([128, 512], FP8, tag="t8")
            nc.vector.tensor_copy(t8[0:96, :], pp[0:96, :])
            dv = dest.rearrange("p (i n) -> p i n", i=2)
            nc.sync.dma_start(out=dv[0:24, :, cs], in_=t8[0:48, :])
            nc.sync.dma_start(out=dv[64:88, :, cs], in_=t8[48:96, :])
            del proj_state[(ns, pair, qk, c4)]

    def emit_proj_window(pair, qk, c4, ns=0):
        for dc in range(DT3):
            emit_proj_step(pair, qk, c4, dc, ns=ns)

    def proj_steps(pair, c4s=range(4)):
        return [
            (pair, qk, c4, dc)
            for c4 in c4s for qk in (0, D) for dc in range(DT3)
        ]

    # --- stage A2 (deferred): v projection for one n-tile ---------------
    def emit_a2(nt):
        ts_ = slice(nt * 128, (nt + 1) * 128)
        pv = scratch.tile([128, D], F32, tag="scr")
        for dc in range(DT3):
            nc.tensor.matmul(
                pv, xT[dc][:, ts_], wqkv_sb[dc][:, 2 * D : 3 * D],
                start=(dc == 0), stop=(dc == DT3 - 1),
            )
        # pv rows = keys of nt; even heads -> v8 slot nt%2 of block nt//2,
        # odd heads -> vb16[nt] bf16
        vph = v8[nt // 2].rearrange("p (hp i c) -> p hp i c", i=2, c=VW)
        vbh = vb16[nt].rearrange("p (hp c) -> p hp c", c=VW)
        pvh = pv.rearrange("p (hp two c) -> p hp two c", two=2, c=DK)
        nc.vector.tensor_copy(vph[:, :, nt % 2, 0:48], pvh[:, :, 0, :])
        nc.vector.tensor_copy(vbh[:, :, 16:64], pvh[:, :, 1, :])

    # --- stage A: x load, transpose, pair-0 projections -----------------
    # Emitted inline for rep 0; for rep r+1 the same items are preloaded
    # into rep r's pair-3 attention (all tiles are shared persists, so
    # only emission order changes).
    def stageA_items(ns, x_engine):
        items = []
        for nt in range(NT):
            def item(nt=nt):
                ts_ = slice(nt * 128, (nt + 1) * 128)
                x_t = xload.tile([128, D], F32, tag="x",
                                 name=f"x_t{rep}_{ns}_{nt}")
                x_engine.dma_start(out=x_t, in_=x[ts_, :])
                xb = xbp.tile([128, D], BF16, tag="xb",
                              name=f"xb{rep}_{ns}_{nt}")
                nc.vector.tensor_copy(xb, x_t)
                tp = scratch.tile([128, D], BF16, tag="scr",
                                  name=f"tp{rep}_{ns}_{nt}")
                for dc in range(DT3):
                    nc.tensor.transpose(
                        tp[:, dc * 128 : (dc + 1) * 128],
                        xb[:, dc * 128 : (dc + 1) * 128],
                        ident,
                    )
                for dc in range(DT3):
                    nc.vector.tensor_copy(
                        xT[dc][:, ts_], tp[:, dc * 128 : (dc + 1) * 128]
                    )
                # pair-0 projections as soon as their xT columns complete
                if "b" in stages and nt % 4 == 3:
                    c4 = nt // 4
                    emit_proj_window(0, 0, c4, ns=ns)
                    emit_proj_window(0, D, c4, ns=ns)
            items.append(item)
        return items

    if "a" in stages and not stageA_preloaded:
        for it in stageA_items(0, nc.scalar if rep == 0 else nc.sync):
            it()
    elif "b" in stages and "a" not in stages:
        for c4 in range(4):
            emit_proj_window(0, 0, c4)
            emit_proj_window(0, D, c4)

    preload_items = (
        stageA_items(1, nc.sync) if (preload_next and "a" in stages) else []
    )

    if deferred is not None:
        deferred()

    # --- correction colsums: cs = (sum_n x) @ W_v ----------------------
    # sxT[:, dc] = row-sums of xT[dc]; cs_v[1, 384] = sxT^T @ W_v chunks;
    # scattered into per-pair [128,1] csp tiles (fp32).  Emitted in pieces
    # interleaved into pair-0's first q-chunk so the DVE reduces don't
    # delay the A2 v8 copies.
    csp = []
    cs_state = {}

    def emit_cs_piece(i):
        if i == 0:
            cs_state["sxT"] = csxpool.tile([128, DT3], F32, tag="sxT", name=f"sxT{rep}")
        if i < DT3:
            nc.vector.tensor_reduce(
                cs_state["sxT"][:, i : i + 1], xT[i], mybir.AxisListType.X,
                OP.add,
            )
            return
        sxb = csxpool.tile([128, DT3], BF16, tag="sxb")
        nc.vector.tensor_copy(sxb, cs_state["sxT"])
        cs_v = scratch.tile([1, D], F32, tag="scr", name=f"csv{rep}")
        for dc in range(DT3):
            nc.tensor.matmul(
                cs_v, sxb[:, dc : dc + 1], wqkv_sb[dc][:, 2 * D : 3 * D],
                start=(dc == 0), stop=(dc == DT3 - 1),
            )
        csrow = csxpool.tile([1, D], F32, tag="csrow")
        nc.vector.tensor_copy(csrow, cs_v)
        for p in range(H // 2):
            ct = cspool.tile([128, 1], F32, tag=f"cs{p}", name=f"cs{p}_{rep}")
            nc.gpsimd.memset(ct, 0.0)
            nc.sync.dma_start(out=ct[0:48, 0:1],
                              in_=csrow[0:1, 96 * p : 96 * p + 48])
            csp.append(ct)

    if "n" in stages and "a" in stages and "s" not in stages:
        for i in range(DT3 + 1):
            emit_cs_piece(i)

    # --- fc_o window (one n-tile) --------------------------------------
    def emit_fc(nt):
        ts_ = slice(nt * 128, (nt + 1) * 128)
        pf = opsum.tile([128, D], F32, tag="o", name=f"pf{rep}_{nt}")
        for p in range(H // 2):
            nc.tensor.matmul(
                pf, attn_p[p][:, ts_], wo_sb[p],
                start=(p == 0), stop=(p == H // 2 - 1),
            )
        ot = fout.tile([128, D], F32, tag="ot")
        nc.vector.tensor_add(ot, pf, b_bcast)
        nc.sync.dma_start(out=out[ts_, :], in_=ot)

    # --- attention ------------------------------------------------------
    tail_fc = []
    if "s" in stages:
        for pair in range(H // 2):
            hA, hB = 2 * pair, 2 * pair + 1
            qp, kp = q_pack[pair], k_pack[pair]
            if "b" in stages and pair == 0:
                pending_proj = proj_steps(1)
            elif "b" in stages and pair == 1:
                # both remaining pairs: frees pairs 2-3 for the cross-rep
                # stage-A preload (xT writes must also come after these)
                pending_proj = proj_steps(2) + proj_steps(3)
            else:
                pending_proj = []
            preload_ctr = 0
            pending_fc = []
            for c5 in range(N // 512):
                cqs = slice(c5 * 512, (c5 + 1) * 512)
                oAB = opsum.tile([128, 512], F32, tag="o")
                pt8 = None
                pend = None
                pendb = None

                def emit_pvb(t_, ptb_):
                    nc.tensor.matmul(
                        oAB[64:128, :],
                        vb16[t_][:, pair * VW : (pair + 1) * VW],
                        ptb_[:, 512:1024],
                        start=(t_ == 0), stop=(t_ == NT - 1),
                        skip_group_check=True,
                    )

                def emit_pv(pend_):
                    b, pt8_ = pend_
                    p8r = pt8_.rearrange("p (i q) -> p i q", i=2)
                    v8v = v8[b].rearrange("p (h i c) -> p h i c",
                                          i=2, c=VW)
                    nc.tensor.matmul(
                        oAB[0:64, :], v8v[:, pair, :, :], p8r,
                        start=(b == 0), stop=(b == NB - 1),
                        perf_mode=DR, skip_group_check=True,
                    )

                kv = kp.rearrange("p (i n) -> p i n", i=2)
                qv = qp.rearrange("p (i n) -> p i n", i=2)
                for t in range(NT):
                    ts_ = slice(t * 128, (t + 1) * 128)
                    sAB = spsum.tile([128, 1024], F32, tag="sAB")
                    nc.tensor.matmul(
                        sAB[:, 0:512], kv[0:24, :, ts_], qv[0:24, :, cqs],
                        start=True, stop=True, perf_mode=DR,
                    )
                    nc.tensor.matmul(
                        sAB[:, 512:1024], kv[64:88, :, ts_], qv[64:88, :, cqs],
                        start=True, stop=True, perf_mode=DR,
                    )
                    if t % 2 == 0 and pending_proj:
                        emit_proj_step(*pending_proj.pop(0))
                        if t % 4 == 0 and pending_proj and pair == 1:
                            emit_proj_step(*pending_proj.pop(0))
                    if pair == 0 and c5 == 0 and "a" in stages:
                        emit_a2(t)
                        if "n" in stages and t in (5, 9, 13, 15):
                            emit_cs_piece({5: 0, 9: 1, 13: 2, 15: 3}[t])
                        if carry_fc and t in (7, 9, 11, 13):
                            carry_fc.pop(0)()
                    if pair >= 2 and t % 2 == 0 and preload_items:
                        preload_ctr += 1
                        if preload_ctr % 3 == 1:
                            preload_items.pop(0)()
                    if pending_fc and t in (3, 5, 7, 9):
                        pending_fc.pop(0)()
                    if "e" not in stages:
                        continue
                    ptb = ptpool.tile([128, 1024], BF16, tag="ptb")
                    nc.scalar.activation(ptb, sAB, AF.Exp, scale=SCALE)
                    if "p" not in stages:
                        continue
                    # Pool: pt8 slot t%2 = (head-A half of ptb) - 1, fp8,
                    # slot-major (DR rhs outer stride must be 16B-aligned)
                    if t % 2 == 0:
                        pt8 = pt8pool.tile([128, 1024], FP8, tag="pt8")
                    nc.gpsimd.tensor_scalar(
                        pt8[:, (t % 2) * 512 : (t % 2) * 512 + 512],
                        ptb[:, 0:512], 1.0, None, OP.subtract
                    )
                    # odd head: plain bf16 PV straight from ptb (pended one
                    # tile so the PE stream stays ahead of ACT)
                    if pendb is not None:
                        emit_pvb(*pendb)
                    pendb = (t, ptb)
                    if t % 2 == 1:
                        if pend is not None:
                            emit_pv(pend)
                        pend = (t // 2, pt8)
                if pendb is not None:
                    emit_pvb(*pendb)
                    pendb = None
                if pend is not None:
                    emit_pv(pend)
                    pend = None

                if "n" not in stages:
                    continue
                # normalization: Z_A row 63 (pt' sums, needs +2048 = the
                # ones-column colsum), Z_B row 64 (exact bf16 sums).
                ztmp = ztpool.tile([64, 512], F32, tag="zt")
                nc.vector.tensor_scalar_add(ztmp[32:64, :], oAB[32:64, :],
                                            float(N))
                zr = zpool.tile([96, 512], F32, tag="zr")
                nc.vector.reciprocal(zr[32:64, :], ztmp[32:64, :])
                nc.vector.reciprocal(zr[64:96, :], oAB[64:96, :])
                # broadcast 1/Z across partitions via a DRAM bounce:
                # row 63 -> zb rows 0:64, row 64 -> zb rows 64:128
                zd = zdpool.tile([2, 512], F32, tag="zd")
                nc.sync.dma_start(out=zd, in_=zr[63:65, :])
                zb = zbpool.tile([128, 512], F32, tag="zb")
                zsrc = bass.AP(
                    tensor=zd.tensor, offset=zd.offset,
                    ap=[[512, 2], [0, 64], [1, 512]],
                )
                nc.sync.dma_start(out=zb, in_=zsrc)
                # attn = (o + cs) * (1/Z): fused add of the exact colsum
                # correction (rows 48:80 of csp are zero).
                nc.vector.scalar_tensor_tensor(
                    attn_p[pair][:, cqs], oAB, csp[pair], zb,
                    op0=OP.add, op1=OP.mult,
                )

                if pair == H // 2 - 1 and "f" in stages:
                    if c5 == 3 and preload_next:
                        tail_fc.extend(
                            (lambda nt=nt: emit_fc(nt))
                            for nt in range(c5 * 4, c5 * 4 + 4)
                        )
                    elif c5 < 3:
                        # defer into the next c5's t-loop: the pf PSUM WAR
                        # on the z-chain otherwise head-of-line blocks PE
                        pending_fc.extend(
                            (lambda nt=nt: emit_fc(nt))
                            for nt in range(c5 * 4, c5 * 4 + 4)
                        )
                    else:
                        for nt in range(c5 * 4, c5 * 4 + 4):
                            emit_fc(nt)
    if carry_fc:
        # shouldn't happen (4 slots for 4 closures), but don't drop work
        for f in carry_fc:
            f()
    return tail_fc


_NC_CACHE = None


def _get_nc():
    global _NC_CACHE
    if _NC_CACHE is None:
        _NC_CACHE = build_nc()
    return _NC_CACHE


def kernel(x, W_qkv, W_o, b_o):
    x = np.asarray(x, dtype=np.float32)
    W_qkv = np.ascontiguousarray(np.asarray(W_qkv, dtype=np.float32))
    W_o = np.ascontiguousarray(np.asarray(W_o, dtype=np.float32))
    b_o = np.ascontiguousarray(np.asarray(b_o, dtype=np.float32))
    b, p, n, d = x.shape
    assert (b, p, n, d) == (NCORES, 1, N, D), x.shape

    nc = _get_nc()
    in_maps = [
        {
            "x": np.ascontiguousarray(x[i, 0]),
            "W_qkv": W_qkv,
            "W_o": W_o,
            "b_o": b_o,
        }
        for i in range(NCORES)
    ]
    res = run_bass_kernel_spmd(nc, in_maps, core_ids=list(range(NCORES)))
    outs = np.stack([res.results[i]["out"] for i in range(NCORES)])
    return outs[:, None].astype(np.float32)
